# revision 46
# baseline (speedup 1.0000x reference)
"""Trainium2 Bass kernel for a transformer decoder layer (self-attn + cross-attn + FFN).

Sharding: 8 cores; cores 0-3 handle batch 0, cores 4-7 batch 1; each core owns a
contiguous 512-token slice of queries.  K/V projections are head-sharded within
each batch group (256 o-dims per core) and AllGathered in fp8; additionally every
core redundantly computes K/V for heads 0-3 so the first two head-pairs of
self-attention run before the AllGather lands.

Precision: fp8e4m3 (+DoubleRow, K=256/matmul) only where the error is softmax-
attenuated — K/V/Q projections (weights x32), Q.K^T scores, exp(), and the AV
matmul.  O-projection, fc1 and fc2 run bf16 (their weight-quantization error
lands directly on the residual stream).  LayerNorm: var = E[x^2]-E[x]^2, rstd =
exp(-0.5*ln(var+eps)) so the scalar engine never leaves the exp/ln table set.
Softmax denominators ride the AV matmul as an appended ones column (output
partition 64); row broadcasts go through GPSIMD partition_broadcast; residual
adds ride PSUM as identity matmuls.  Queries run in two 256-column chunks so
chunk-0 FFN overlaps chunk-1 exps on the scalar engine.
"""

import os
import sys
import time

for _p in ("/opt/trn_rl_repo", "/root/.axon_site/_ro/trn_rl_repo"):
    if _p not in sys.path:
        sys.path.append(_p)

import numpy as np
import ml_dtypes

B, T, D, H, DH, FFN = 2, 2048, 1024, 16, 64, 4096
N_CORES = 8
CPB = N_CORES // B          # cores per batch
TL = T // CPB               # local tokens per core
DC = D // 128               # d-chunks (8)
KC = T // 128               # key chunks (16)
OC1 = FFN // 128            # fc1 out chunks (32)
NT = T // 512               # 512-wide column tiles over T
EPS = 1e-12
SCALE = 1.0 / 8.0           # 1/sqrt(DH)
MASK_NEG = -80000.0         # additive mask value (pre-scale)

W = 256                     # query chunk width
NCH = TL // W               # chunks (2)
VP = 80                     # padded V free width (64 + den col + pad, %16==0)

WS = 32.0                   # q/k/v weight fp8 pre-scale

USE_DR = not os.environ.get("BASS_NO_DR")
DEBUG_DUMP = bool(os.environ.get("BASS_DEBUG_DUMP"))

bf16 = ml_dtypes.bfloat16
f8 = ml_dtypes.float8_e4m3

_CACHE = {}

KSLICE = 2 * 128            # per-core K/V head-dim slice
KPART = KSLICE * T
VPART = T * KSLICE
AGLEN = KPART + VPART


def _emit(ctx, tc, nc, aps, use_mask, dbg=None):
    import concourse.bass as bass
    from concourse import mybir
    from contextlib import ExitStack
    dt = mybir.dt
    AF = mybir.ActivationFunctionType
    Alu = mybir.AluOpType
    PM = mybir.MatmulPerfMode
    fp32 = dt.float32
    bf = dt.bfloat16
    e4 = dt.float8e4

    consts = ctx.enter_context(tc.tile_pool(name="consts", bufs=1))
    sbA = ctx.enter_context(tc.tile_pool(name="sbA", bufs=1))
    smallp = ctx.enter_context(tc.tile_pool(name="smallp", bufs=4))
    bcp = ctx.enter_context(tc.tile_pool(name="bcp", bufs=4))
    lnp = ctx.enter_context(tc.tile_pool(name="lnp", bufs=1))
    ps_lin = ctx.enter_context(tc.tile_pool(name="ps_lin", bufs=2, space="PSUM"))
    ps_sc = ctx.enter_context(tc.tile_pool(name="ps_sc", bufs=2, space="PSUM"))
    ps_av = ctx.enter_context(tc.tile_pool(name="ps_av", bufs=2, space="PSUM"))

    # ---- constants ----
    ones_bf = consts.tile([1, 512], bf)
    nc.gpsimd.memset(ones_bf[:], 1.0)
    ones_bfc = consts.tile([128, 1], bf)
    nc.gpsimd.memset(ones_bfc[:], 1.0)
    idt = consts.tile([128, 128], bf, tag="idt")
    nc.sync.dma_start(idt[:], aps["id_t"][:])
    eps_t = consts.tile([1, 1], fp32)
    nc.gpsimd.memset(eps_t[:], EPS)

    # packed bias rows (q/k/v pre-scaled x32 on host; o/fc at true scale)
    NB = 5 * D + 6 * KSLICE
    batile = consts.tile([1, NB], bf, tag="batile")
    nc.sync.dma_start(batile[:], aps["biases_att"][:])
    _off = {}
    _o = 0
    for nm, ln in (("b_saq", D), ("b_sao", D), ("b_caq", D), ("b_cao", D),
                   ("b2r", D), ("b_sak", KSLICE), ("b_sav", KSLICE),
                   ("b_cak", KSLICE), ("b_cav", KSLICE),
                   ("b_sak0", KSLICE), ("b_sav0", KSLICE)):
        _off[nm] = (_o, ln)
        _o += ln
    bias_rows = {nm: batile[:, o:o + ln] for nm, (o, ln) in _off.items()}

    gbe_t = consts.tile([128, 48], fp32, tag="gbe")
    nc.sync.dma_start(gbe_t[:], aps["gbe_all"][:])

    def gbe(i):
        return gbe_t[:, i * DC:(i + 1) * DC]

    # =========================== helpers ===========================

    def mm2(p_out, lhsT3, rhs3, start, stop):
        """One fp8 DoubleRow matmul (or two plain ones when disabled)."""
        if USE_DR:
            nc.tensor.matmul(p_out, lhsT3, rhs3, start=start, stop=stop,
                             perf_mode=PM.DoubleRow)
        else:
            nc.tensor.matmul(p_out, lhsT3[:, 0], rhs3[:, 0],
                             start=start, stop=False)
            nc.tensor.matmul(p_out, lhsT3[:, 1], rhs3[:, 1],
                             start=False, stop=stop)

    def qproj_chain(p_out, w_sb, oc, x_sb, xsl, b_row):
        """fp8 q-projection: 4 DR matmuls + rank-1 bias."""
        for j in range(DC // 2):
            mm2(p_out, w_sb[:, 2 * j:2 * j + 2, oc * 128:(oc + 1) * 128],
                x_sb[:, 2 * j:2 * j + 2, xsl], start=(j == 0), stop=False)
        nc.tensor.matmul(p_out, b_row[:, oc * 128:(oc + 1) * 128],
                         ones_bf[:, 0:p_out.shape[-1]], start=False, stop=True)

    # =========================== K/V ===========================

    agp = ctx.enter_context(tc.tile_pool(name="agp", bufs=1, space="DRAM"))
    drp = ctx.enter_context(tc.tile_pool(name="drp", bufs=2, space="DRAM"))
    kvw = ctx.enter_context(tc.tile_pool(name="kvw", bufs=2))
    stgp = ctx.enter_context(tc.tile_pool(name="stgp", bufs=2))
    srcp = ctx.enter_context(tc.tile_pool(name="srcp", bufs=2))

    def kv_proj(src_name, wk_name, bk, wv_name, bv, k_sink, v_sink):
        """Stream src columns; project K (feature-major) and V (token-major)."""
        wk = kvw.tile([128, DC, KSLICE], e4, tag="wkv")
        nc.sync.dma_start(wk[:], aps[wk_name].rearrange("(c p) o -> p c o", p=128))
        wv = kvw.tile([128, DC, KSLICE], e4, tag="wkv")
        nc.sync.dma_start(wv[:], aps[wv_name].rearrange("(c p) o -> p c o", p=128))
        src_ap = aps[src_name].rearrange("(c p) t -> p c t", p=128)
        for nt in range(NT):
            src = srcp.tile([128, DC, 512], e4, tag="src")
            nc.sync.dma_start(src[:], src_ap[:, :, nt * 512:(nt + 1) * 512])
            for oc in range(2):
                p = ps_lin.tile([128, 512], fp32, tag="lin")
                for j in range(DC // 2):
                    mm2(p[:], wk[:, 2 * j:2 * j + 2, oc * 128:(oc + 1) * 128],
                        src[:, 2 * j:2 * j + 2, :], start=(j == 0), stop=False)
                nc.tensor.matmul(p[:], bias_rows[bk][:, oc * 128:(oc + 1) * 128],
                                 ones_bf[:], start=False, stop=True)
                k_sink(oc, nt, p)
            for k4 in range(4):
                kc = 4 * nt + k4
                p = ps_lin.tile([128, 512], fp32, tag="lin")
                for j in range(DC // 2):
                    mm2(p[:, 0:KSLICE],
                        src[:, 2 * j:2 * j + 2, k4 * 128:(k4 + 1) * 128],
                        wv[:, 2 * j:2 * j + 2, :], start=(j == 0), stop=False)
                nc.tensor.matmul(p[:, 0:KSLICE], ones_bf[:, 0:128],
                                 bias_rows[bv][:], start=False, stop=True)
                v_sink(kc, p)

    def kv_part(src_name, wk_name, bk, wv_name, bv):
        """This core's K/V head-slice (fp8) -> AllGather."""
        ag_in = agp.tile([AGLEN], e4, tag="ag_in")
        k_reg = ag_in[0:KPART].rearrange("(oc p t) -> oc p t", oc=2, p=128)
        v_reg = ag_in[KPART:AGLEN].rearrange("(kc p o) -> kc p o", kc=KC, p=128)

        def k_sink(oc, nt, p):
            stg = stgp.tile([128, 512], e4, tag="stg")
            nc.scalar.activation(out=stg[:], in_=p[:], func=AF.Copy,
                                 scale=1.0 / WS)
            nc.sync.dma_start(k_reg[oc, :, nt * 512:(nt + 1) * 512], stg[:])

        def v_sink(kc, p):
            stg = stgp.tile([128, 512], e4, tag="stg")
            nc.scalar.activation(out=stg[:, 0:KSLICE], in_=p[:, 0:KSLICE],
                                 func=AF.Copy, scale=1.0 / WS)
            nc.sync.dma_start(v_reg[kc], stg[:, 0:KSLICE])

        kv_proj(src_name, wk_name, bk, wv_name, bv, k_sink, v_sink)
        ag_out = agp.tile([CPB, AGLEN], e4, tag="ag_out")
        nc.gpsimd.collective_compute(
            "AllGather", Alu.bypass,
            ins=[ag_in.opt()], outs=[ag_out.opt()],
            replica_groups=[list(range(CPB)), list(range(CPB, 2 * CPB))])
        return ag_out

    def local_kv(src_name, wk_name, bk, wv_name, bv, lkv):
        """K/V for heads 0-3 computed locally on every core (covers pr 0,1)."""
        lktp = lkv.tile([128, 2, T], e4, tag="lktp")
        lva = lkv.tile([128, KC, 4, VP], e4, tag="lva")
        nc.gpsimd.memset(lva[:, :, :, 64:65], 1.0)
        nc.gpsimd.memset(lva[:, :, :, 65:VP], 0.0)

        def k_sink(oc, nt, p):
            nc.scalar.activation(out=lktp[:, oc, nt * 512:(nt + 1) * 512],
                                 in_=p[:], func=AF.Copy, scale=1.0 / WS)

        def v_sink(kc, p):
            nc.scalar.activation(
                out=lva[:, kc, :, 0:64],
                in_=p[:, 0:KSLICE].rearrange("p (h d) -> p h d", h=4),
                func=AF.Copy, scale=1.0 / WS)

        kv_proj(src_name, wk_name, bk, wv_name, bv, k_sink, v_sink)
        return lktp, lva

    # =========================== attention ===========================

    ktpp = ctx.enter_context(tc.tile_pool(name="ktpp", bufs=2))
    vap = ctx.enter_context(tc.tile_pool(name="vap", bufs=2))
    expp = ctx.enter_context(tc.tile_pool(name="expp", bufs=2))

    def attn_pr(qt, pr, c, ag_out, lkv_pair, mask_sb, praw, den_all, dbg=None):
        """One (head-pair, query-chunk): QK^T, exp, AV; stage raw AV + denoms."""
        csl = slice(c * W, (c + 1) * W)
        if lkv_pair is not None:
            lktp, lva = lkv_pair
            ktp_t = lktp[:, pr, :]
            va_of = {hf: lva[:, :, 2 * pr + hf, :] for hf in range(2)}
        else:
            ktp_t = ktpp.tile([128, T], e4, tag="ktp")
            k_src = ag_out[pr // 2, 0:KPART].rearrange(
                "(oc p t) -> oc p t", oc=2, p=128)[pr % 2]
            nc.sync.dma_start(ktp_t[:], k_src)
            va_of = {}
            for hf in range(2):
                h = 2 * pr + hf
                va_h = vap.tile([128, KC, VP], e4, tag="vah")
                nc.gpsimd.memset(va_h[:, :, 64:65], 1.0)
                nc.gpsimd.memset(va_h[:, :, 65:VP], 0.0)
                v_src = ag_out[h // 4, KPART:AGLEN].rearrange(
                    "(kc p hl d) -> p kc hl d", kc=KC, p=128, hl=4)
                nc.sync.dma_start(va_h[:, :, 0:64], v_src[:, :, h % 4, :])
                va_of[hf] = va_h
        exp_t = expp.tile([128, 8, 2, 2, W], e4, tag="exp")
        for g in range(8):
            p = ps_sc.tile([128, 2, 2, W], fp32, tag="sc")
            for j2 in range(2):
                kc = 2 * g + j2
                for hf in range(2):
                    po = 64 * hf
                    nc.tensor.matmul(
                        p[:, hf, j2, :],
                        ktp_t[po:po + 64, kc * 128:(kc + 1) * 128],
                        qt[po:po + 64, pr, csl], start=True, stop=True)
                    if mask_sb is not None:
                        nc.vector.tensor_add(
                            out=p[:, hf, j2, :], in0=p[:, hf, j2, :],
                            in1=mask_sb[:, kc, csl])
            nc.scalar.activation(out=exp_t[:, g], in_=p[:], func=AF.Exp,
                                 scale=SCALE)
        if dbg is not None:
            nc.sync.dma_start(dbg["d_exp"],
                              exp_t.rearrange("p g a b w -> p (g a b w)"))
        for hf in range(2):
            pav_t = ps_av.tile([128, 512], fp32, tag="av")
            pav = pav_t[0:VP, 0:W]
            for g in range(8):
                mm2(pav, va_of[hf][:, 2 * g:2 * g + 2, :], exp_t[:, g, hf],
                    start=(g == 0), stop=(g == 7))
            h = 2 * pr + hf
            po = 64 * hf
            nc.vector.tensor_copy(out=praw[po:po + 64, :], in_=pav[0:64, :])
            dtmp = smallp.tile([1, W], fp32, tag="dtmp", bufs=4)
            nc.vector.tensor_copy(out=dtmp[:], in_=pav[64:65, :])
            nc.sync.dma_start(den_all[h:h + 1, :], dtmp[:])
            if dbg is not None and hf == 0:
                dpav = smallp.tile([128, W], fp32, tag="dpav", bufs=1)
                nc.vector.tensor_copy(out=dpav[0:VP, :], in_=pav[:])
                nc.sync.dma_start(dbg["d_pav"], dpav[:])

    def finish_attn(praws, den_all, attn_dst, dbg=None):
        """Batched softmax normalize: one reciprocal for all 16 heads."""
        rec = smallp.tile([16, W], bf, tag="rec", bufs=2)
        with nc.allow_low_precision(reason="softmax denominators are O(1e3)"):
            nc.vector.reciprocal(out=rec[:], in_=den_all[:])
        rdr = drp.tile([16, W], bf, tag="rdr")
        nc.sync.dma_start(rdr[:], rec[:])
        for pr in range(DC):
            rb = bcp.tile([128, W], bf, tag="bc")
            nc.sync.dma_start(rb[0:64, :],
                              rdr[2 * pr:2 * pr + 1, :].to_broadcast([64, W]))
            nc.sync.dma_start(rb[64:128, :],
                              rdr[2 * pr + 1:2 * pr + 2, :].to_broadcast([64, W]))
            if dbg is not None and pr == 0:
                nc.sync.dma_start(dbg["d_rrow"], rec[0:1, :])
                nc.sync.dma_start(dbg["d_rb"], rb[0:64, :])
            nc.vector.tensor_mul(out=attn_dst[:, pr, :],
                                 in0=praws[pr][:], in1=rb[:])

    # ======================= bf16 linears =======================

    wop = ctx.enter_context(tc.tile_pool(name="wop", bufs=2))

    def oproj(attn_c, wname, b_o, res_bf, res_sl, c):
        """bf16 O-projection + bias + residual (identity matmul) -> y bf16."""
        w_ap = aps[wname].rearrange("(c p) o -> p c o", p=128)
        y = lnp.tile([128, DC, W], bf, tag="y", bufs=2)
        for oc in range(DC):
            wo_oc = wop.tile([128, DC, 128], bf, tag="wo")
            nc.sync.dma_start(wo_oc[:], w_ap[:, :, oc * 128:(oc + 1) * 128])
            p = ps_lin.tile([128, 512], fp32, tag="lin")
            pw = p[:, 0:W]
            for dc in range(DC):
                nc.tensor.matmul(pw, wo_oc[:, dc, :], attn_c[:, dc, :],
                                 start=(dc == 0), stop=False)
            nc.tensor.matmul(pw, b_o[:, oc * 128:(oc + 1) * 128],
                             ones_bf[:, 0:W], start=False, stop=False)
            nc.tensor.matmul(pw, idt[:], res_bf[:, oc, res_sl],
                             start=False, stop=True)
            nc.scalar.activation(out=y[:, oc, :], in_=pw, func=AF.Copy,
                                 scale=1.0)
        return y

    def layernorm(y, gi, out_bf, out_f8, out_dma):
        """LN over d: var = E[x^2]-E[x]^2, rstd = exp(-0.5*ln(var+eps))."""
        g, be = gbe(2 * gi), gbe(2 * gi + 1)
        pm_t = ps_lin.tile([128, 512], fp32, tag="lin")
        pv_t = ps_lin.tile([128, 512], fp32, tag="lin")
        pm, pv = pm_t[0:1, 0:W], pv_t[0:1, 0:W]
        for dc in range(DC):
            ysq = lnp.tile([128, W], bf, tag="ysq", bufs=3)
            nc.vector.tensor_mul(out=ysq[:], in0=y[:, dc, :], in1=y[:, dc, :])
            nc.tensor.matmul(pm, ones_bfc[:], y[:, dc, :],
                             start=(dc == 0), stop=(dc == DC - 1))
            nc.tensor.matmul(pv, ones_bfc[:], ysq[:],
                             start=(dc == 0), stop=(dc == DC - 1))
        mrow = smallp.tile([1, W], fp32, tag="row")
        nc.vector.tensor_scalar(out=mrow[:], in0=pm, scalar1=1.0 / D,
                                scalar2=None, op0=Alu.mult)
        msq = smallp.tile([1, W], fp32, tag="row")
        nc.vector.tensor_scalar(out=msq[:], in0=pv, scalar1=1.0 / D,
                                scalar2=None, op0=Alu.mult)
        var = smallp.tile([1, W], fp32, tag="row")
        nc.vector.tensor_mul(out=var[:], in0=mrow[:], in1=mrow[:])
        nc.vector.tensor_sub(out=var[:], in0=msq[:], in1=var[:])
        lnv = smallp.tile([1, W], fp32, tag="row")
        nc.scalar.activation(out=lnv[:], in_=var[:], func=AF.Ln, bias=eps_t[:])
        rstd = smallp.tile([1, W], bf, tag="rowb")
        nc.scalar.activation(out=rstd[:], in_=lnv[:], func=AF.Exp, scale=-0.5)
        mbf = smallp.tile([1, W], bf, tag="rowb")
        nc.vector.tensor_copy(out=mbf[:], in_=mrow[:])
        rb = bcp.tile([128, W], bf, tag="bcb")
        nc.gpsimd.partition_broadcast(rb[:], rstd[:])
        mb = bcp.tile([128, W], bf, tag="bcb")
        nc.gpsimd.partition_broadcast(mb[:], mbf[:])
        for dc in range(DC):
            t0 = lnp.tile([128, W], bf, tag="t0", bufs=3)
            nc.vector.tensor_sub(out=t0[:], in0=y[:, dc, :], in1=mb[:])
            nc.vector.tensor_mul(out=t0[:], in0=t0[:], in1=rb[:])
            if out_dma is not None:
                od = lnp.tile([128, W], fp32, tag="od", bufs=2)
                nc.vector.tensor_scalar(
                    out=od[:], in0=t0[:],
                    scalar1=g[:, dc:dc + 1], scalar2=be[:, dc:dc + 1],
                    op0=Alu.mult, op1=Alu.add)
                nc.sync.dma_start(out_dma[:, dc, :], od[:])
            else:
                nc.vector.tensor_scalar(
                    out=out_bf[:, dc, :], in0=t0[:],
                    scalar1=g[:, dc:dc + 1], scalar2=be[:, dc:dc + 1],
                    op0=Alu.mult, op1=Alu.add)
                if out_f8 is not None:
                    nc.vector.tensor_scalar(
                        out=out_f8[:, dc, :], in0=t0[:],
                        scalar1=g[:, dc:dc + 1], scalar2=be[:, dc:dc + 1],
                        op0=Alu.mult, op1=Alu.add)

    # =========================== pipeline ===========================

    with ExitStack() as attn_ctx:
        lkv = attn_ctx.enter_context(tc.tile_pool(name="lkv", bufs=1))
        wqp = attn_ctx.enter_context(tc.tile_pool(name="wqp", bufs=2))

        ag_sa = kv_part("decT_f8", "w_sak", "b_sak", "w_sav", "b_sav")
        do_local = not use_mask
        lkv_sa = (local_kv("decT_f8", "w_sak0", "b_sak0", "w_sav0", "b_sav0",
                           lkv) if do_local else None)
        ag_ca = kv_part("encT_f8", "w_cak", "b_cak", "w_cav", "b_cav")

        xloc_bf = sbA.tile([128, DC, TL], bf, tag="xlbf")
        nc.sync.dma_start(xloc_bf[:],
                          aps["xlocT_bf"].rearrange("(c p) t -> p c t", p=128))
        xloc_f8 = sbA.tile([128, DC, TL], e4, tag="xlf8")
        nc.sync.dma_start(xloc_f8[:],
                          aps["xlocT_f8"].rearrange("(c p) t -> p c t", p=128))

        mask_sa = mask_ca = None
        if use_mask:
            mask_sa = sbA.tile([128, KC, TL], bf, tag="mask", name="mask_sa")
            nc.sync.dma_start(mask_sa[:],
                              aps["amask_saT"].rearrange("(c p) t -> p c t", p=128))

        # SA Q projection (fp8, full width)
        wq = wqp.tile([128, DC, D], e4, tag="wq")
        nc.sync.dma_start(wq[:], aps["w_saq"].rearrange("(c p) o -> p c o", p=128))
        qt_sa = sbA.tile([128, DC, TL], e4, tag="qt")
        for oc in range(DC):
            p = ps_lin.tile([128, 512], fp32, tag="lin")
            qproj_chain(p[:], wq, oc, xloc_f8, slice(None), bias_rows["b_saq"])
            nc.scalar.activation(out=qt_sa[:, oc, :], in_=p[:], func=AF.Copy,
                                 scale=1.0 / WS)
        if dbg is not None:
            nc.sync.dma_start(dbg["d_qt"],
                              qt_sa.rearrange("p c t -> p (c t)"))
            if lkv_sa is not None:
                nc.sync.dma_start(dbg["d_ktp"],
                                  lkv_sa[0].rearrange("p c t -> p (c t)"))

        attn_sa = [sbA.tile([128, DC, W], bf, tag=f"attn{c}", name=f"attnsa{c}")
                   for c in range(NCH)]
        den_sa = [smallp.tile([16, W], fp32, tag="den", bufs=4,
                              name=f"densa{c}") for c in range(NCH)]
        praw_sa = [[None] * DC for _ in range(NCH)]

        def new_praw(store, c, pr):
            t = lnp.tile([128, W], bf, tag="praw", bufs=11,
                         name=f"praw{c}_{pr}")
            store[c][pr] = t
            return t

        if do_local:
            for pr in range(2):
                for c in range(NCH):
                    attn_pr(qt_sa, pr, c, None, lkv_sa, mask_sa,
                            new_praw(praw_sa, c, pr), den_sa[c],
                            dbg=(dbg if (pr == 0 and c == 0) else None))

        wqc = wqp.tile([128, DC, D], e4, tag="wq")
        nc.sync.dma_start(wqc[:], aps["w_caq"].rearrange("(c p) o -> p c o", p=128))
        qt_ca = sbA.tile([128, DC, TL], e4, tag="qtc")
        x1_bf = [sbA.tile([128, DC, W], bf, tag=f"x1b{c}", name=f"x1b{c}")
                 for c in range(NCH)]
        x1_f8 = [sbA.tile([128, DC, W], e4, tag=f"x1f{c}", name=f"x1f{c}")
                 for c in range(NCH)]

        pr0 = 0 if not do_local else 2
        for c in range(NCH):
            for pr in range(pr0, DC):
                attn_pr(qt_sa, pr, c, ag_sa, None, mask_sa,
                        new_praw(praw_sa, c, pr), den_sa[c])
            finish_attn(praw_sa[c], den_sa[c], attn_sa[c],
                        dbg=(dbg if c == 0 else None))
            y1 = oproj(attn_sa[c], "w_sao", bias_rows["b_sao"], xloc_bf,
                       slice(c * W, (c + 1) * W), c)
            if dbg is not None and c == 0:
                nc.sync.dma_start(dbg["d_attn"],
                                  attn_sa[0].rearrange("p c t -> p (c t)"))
                nc.sync.dma_start(dbg["d_y1"],
                                  y1.rearrange("p c t -> p (c t)"))
            layernorm(y1, 0, x1_bf[c], x1_f8[c], None)
            if dbg is not None and c == 0:
                nc.sync.dma_start(dbg["d_x1"],
                                  x1_bf[0].rearrange("p c t -> p (c t)"))
            for oc in range(DC):
                p = ps_lin.tile([128, 512], fp32, tag="lin")
                qproj_chain(p[:, 0:W], wqc, oc, x1_f8[c], slice(None),
                            bias_rows["b_caq"])
                nc.scalar.activation(out=qt_ca[:, oc, c * W:(c + 1) * W],
                                     in_=p[:, 0:W], func=AF.Copy, scale=1.0 / WS)

        # ---- cross attention + FFN, chunk-pipelined ----
        if use_mask:
            mask_ca = sbA.tile([128, KC, TL], bf, tag="mask", name="mask_ca")
            nc.sync.dma_start(mask_ca[:],
                              aps["amask_caT"].rearrange("(c p) t -> p c t", p=128))
        w1p = attn_ctx.enter_context(tc.tile_pool(name="w1p", bufs=4))
        w2p = attn_ctx.enter_context(tc.tile_pool(name="w2p", bufs=2))
        w1_ap = aps["w1t"].rearrange("(c p) o -> p c o", p=128)
        w2_ap = aps["w2t"].rearrange("(c p) o -> p c o", p=128)
        b1r = consts.tile([1, FFN], bf, tag="b1r")
        nc.sync.dma_start(b1r[:], aps["b1r"][:])

        attn_ca = [sbA.tile([128, DC, W], bf, tag=f"attn{c}", name=f"attnca{c}")
                   for c in range(NCH)]
        x2_bf = [sbA.tile([128, DC, W], bf, tag=f"x2b{c}", name=f"x2b{c}")
                 for c in range(NCH)]
        out_ap = aps["outT"].rearrange("(c p) t -> p c t", p=128)

        def ffn_chunk(c):
            ht = lnp.tile([128, OC1, W], bf, tag="ht", bufs=1)
            for oc in range(OC1):
                w1oc = w1p.tile([128, DC, 128], bf, tag="w1")
                nc.sync.dma_start(w1oc[:], w1_ap[:, :, oc * 128:(oc + 1) * 128])
                p = ps_lin.tile([128, 512], fp32, tag="lin")
                pw = p[:, 0:W]
                for dc in range(DC):
                    nc.tensor.matmul(pw, w1oc[:, dc, :], x2_bf[c][:, dc, :],
                                     start=(dc == 0), stop=False)
                nc.tensor.matmul(pw, b1r[:, oc * 128:(oc + 1) * 128],
                                 ones_bf[:, 0:W], start=False, stop=True)
                nc.scalar.activation(out=ht[:, oc, :], in_=pw, func=AF.Relu,
                                     scale=1.0)
            y3 = lnp.tile([128, DC, W], bf, tag="y", bufs=2)
            for oc in range(DC):
                for half in range(2):
                    w2oc = w2p.tile([128, OC1 // 2, 128], bf, tag="w2")
                    nc.sync.dma_start(
                        w2oc[:],
                        w2_ap[:, half * 16:(half + 1) * 16,
                              oc * 128:(oc + 1) * 128])
                    if half == 0:
                        p = ps_lin.tile([128, 512], fp32, tag="lin")
                        pw = p[:, 0:W]
                    for j in range(OC1 // 2):
                        nc.tensor.matmul(
                            pw, w2oc[:, j, :], ht[:, half * 16 + j, :],
                            start=(half == 0 and j == 0), stop=False)
                nc.tensor.matmul(pw, bias_rows["b2r"][:, oc * 128:(oc + 1) * 128],
                                 ones_bf[:, 0:W], start=False, stop=False)
                nc.tensor.matmul(pw, idt[:], x2_bf[c][:, oc, :],
                                 start=False, stop=True)
                nc.scalar.activation(out=y3[:, oc, :], in_=pw, func=AF.Copy,
                                     scale=1.0)
            layernorm(y3, 2, None, None, out_ap[:, :, c * W:(c + 1) * W])

        den_ca = [smallp.tile([16, W], fp32, tag="den", bufs=4,
                              name=f"denca{c}") for c in range(NCH)]
        praw_ca = [[None] * DC for _ in range(NCH)]
        for c in range(NCH):
            for pr in range(DC):
                attn_pr(qt_ca, pr, c, ag_ca, None, mask_ca,
                        new_praw(praw_ca, c, pr), den_ca[c])
            finish_attn(praw_ca[c], den_ca[c], attn_ca[c])
            if c == 0:
                y2 = oproj(attn_ca[c], "w_cao", bias_rows["b_cao"], x1_bf[c],
                           slice(None), c)
                layernorm(y2, 1, x2_bf[c], None, None)
        # FFN chunk 0 overlaps CA chunk 1 exps
        ffn_chunk(0)
        y2 = oproj(attn_ca[1], "w_cao", bias_rows["b_cao"], x1_bf[1],
                   slice(None), 1)
        layernorm(y2, 1, x2_bf[1], None, None)
        ffn_chunk(1)


def _build(use_mask):
    import concourse.bass as bass
    import concourse.tile as tile
    from concourse import bacc, mybir
    dt = mybir.dt
    nc = bacc.Bacc("TRN2", target_bir_lowering=False, debug=False,
                   num_devices=N_CORES)
    aps = {}

    def inp(name, shape, dtype):
        aps[name] = nc.dram_tensor(name, shape, dtype, kind="ExternalInput").ap()

    inp("decT_f8", [D, T], dt.float8e4)
    inp("encT_f8", [D, T], dt.float8e4)
    inp("xlocT_bf", [D, TL], dt.bfloat16)
    inp("xlocT_f8", [D, TL], dt.float8e4)
    for nm in ("w_saq", "w_caq"):
        inp(nm, [D, D], dt.float8e4)
    for nm in ("w_sao", "w_cao"):
        inp(nm, [D, D], dt.bfloat16)
    for nm in ("w_sak", "w_sav", "w_cak", "w_cav", "w_sak0", "w_sav0"):
        inp(nm, [D, KSLICE], dt.float8e4)
    inp("w1t", [D, FFN], dt.bfloat16)
    inp("w2t", [FFN, D], dt.bfloat16)
    inp("biases_att", [1, 5 * D + 6 * KSLICE], dt.bfloat16)
    inp("b1r", [1, FFN], dt.bfloat16)
    inp("gbe_all", [128, 48], dt.float32)
    inp("id_t", [128, 128], dt.bfloat16)
    if use_mask:
        inp("amask_saT", [T, TL], dt.bfloat16)
        inp("amask_caT", [T, TL], dt.bfloat16)
    aps["outT"] = nc.dram_tensor("outT", [D, TL], dt.float32,
                                 kind="ExternalOutput").ap()
    dbg = None
    if DEBUG_DUMP:
        dbg = {}
        for nm, shape, dty in (
                ("d_qt", [128, DC * TL], dt.float8e4),
                ("d_ktp", [128, 2 * T], dt.float8e4),
                ("d_exp", [128, 8 * 4 * W], dt.float8e4),
                ("d_pav", [128, W], dt.float32),
                ("d_rrow", [1, W], dt.bfloat16),
                ("d_rb", [64, W], dt.bfloat16),
                ("d_attn", [128, DC * W], dt.bfloat16),
                ("d_y1", [128, DC * W], dt.bfloat16),
                ("d_x1", [128, DC * W], dt.bfloat16)):
            dbg[nm] = nc.dram_tensor(nm, shape, dty, kind="ExternalOutput").ap()

    from contextlib import ExitStack
    with tile.TileContext(nc) as tc:
        with ExitStack() as ctx:
            _emit(ctx, tc, nc, aps, use_mask, dbg=dbg)
    nc.compile()
    return nc


def _make_runner(nc):
    import jax
    from jax.sharding import Mesh, PartitionSpec
    from jax.experimental.shard_map import shard_map
    from concourse import bass2jax, mybir

    bass2jax.install_neuronx_cc_hook()
    part_name = nc.partition_id_tensor.name if nc.partition_id_tensor else None
    in_names, out_names, out_avals = [], [], []
    for alloc in nc.m.functions[0].allocations:
        if not isinstance(alloc, mybir.MemoryLocationSet):
            continue
        name = alloc.memorylocations[0].name
        if alloc.kind == "ExternalInput":
            if name != part_name:
                in_names.append(name)
        elif alloc.kind == "ExternalOutput":
            out_names.append(name)
            out_avals.append(jax.core.ShapedArray(tuple(alloc.tensor_shape),
                                                  mybir.dt.np(alloc.dtype)))
    n_params = len(in_names)
    all_names = list(in_names + out_names)
    if part_name is not None:
        all_names.append(part_name)
    all_names = tuple(all_names)

    def _body(*args):
        operands = list(args)
        if part_name is not None:
            operands.append(bass2jax.partition_id_tensor())
        return tuple(bass2jax._bass_exec_p.bind(
            *operands, out_avals=tuple(out_avals), in_names=all_names,
            out_names=tuple(out_names), lowering_input_output_aliases=(),
            sim_require_finite=True, sim_require_nnan=True, nc=nc))

    devices = jax.devices()[:N_CORES]
    mesh = Mesh(np.asarray(devices), ("core",))
    spec = (PartitionSpec("core"),)
    nin = n_params + len(out_names)
    donate = () if os.environ.get("BASS_KERNEL_SIM") else tuple(
        range(n_params, nin))
    jfn = jax.jit(
        shard_map(_body, mesh=mesh, in_specs=spec * nin,
                  out_specs=spec * len(out_names), check_rep=False),
        donate_argnums=donate, keep_unused=True)

    from jax.sharding import NamedSharding
    shard = NamedSharding(mesh, PartitionSpec("core"))

    def run(in_maps, timeit=False):
        concat_in = [np.concatenate([np.asarray(in_maps[c][n]) for c in range(N_CORES)],
                                    axis=0) for n in in_names]
        zeros = [np.zeros((N_CORES * a.shape[0],) + tuple(a.shape[1:]), a.dtype)
                 for a in out_avals]
        dev_in = [jax.device_put(a, shard) for a in concat_in]
        out = jfn(*dev_in, *[jax.device_put(z, shard) for z in zeros])
        jax.block_until_ready(out)
        times = []
        if timeit:
            for _rep in range(2):
                totals = {}
                for k in (4, 44):
                    zsets = [[jax.device_put(z, shard) for z in zeros]
                             for _ in range(k)]
                    jax.block_until_ready(zsets)
                    t0 = time.perf_counter()
                    outs = [jfn(*dev_in, *zs) for zs in zsets]
                    jax.block_until_ready(outs)
                    totals[k] = time.perf_counter() - t0
                times.append((totals[44] - totals[4]) / 40.0)
        per_core = [{n: np.asarray(out[i]).reshape(N_CORES, *out_avals[i].shape)[c]
                     for i, n in enumerate(out_names)} for c in range(N_CORES)]
        return per_core, times

    return run


def _prep_inputs(dec, enc, t_mask, s_mask, weights):
    use_mask_sa = not bool(np.all(t_mask != 0))
    use_mask_ca = not bool(np.all(s_mask != 0))
    use_mask = use_mask_sa or use_mask_ca

    kvT = weights["_kvT"]
    bias = weights["_bias"]
    shared = {k: v for k, v in weights.items() if not k.startswith("_")}
    in_maps = []
    for c in range(N_CORES):
        b, s = c // CPB, c % CPB
        rows = slice(s * TL, (s + 1) * TL)
        osl = slice(s * KSLICE, (s + 1) * KSLICE)
        m = dict(shared)
        for nm in ("w_sak", "w_sav", "w_cak", "w_cav"):
            m[nm] = np.ascontiguousarray(kvT[nm][:, osl])
        batt = np.concatenate([
            bias["b_saq"] * WS, bias["b_sao"], bias["b_caq"] * WS,
            bias["b_cao"], bias["b2r"],
            bias["b_sak"][osl] * WS, bias["b_sav"][osl] * WS,
            bias["b_cak"][osl] * WS, bias["b_cav"][osl] * WS,
            bias["b_sak"][0:KSLICE] * WS, bias["b_sav"][0:KSLICE] * WS])
        m["biases_att"] = batt.reshape(1, -1).astype(bf16)
        decT = np.ascontiguousarray(dec[b].T)
        m["decT_f8"] = decT.astype(f8)
        m["encT_f8"] = np.ascontiguousarray(enc[b].T).astype(f8)
        xloc = np.ascontiguousarray(dec[b, rows].T)
        m["xlocT_bf"] = xloc.astype(bf16)
        m["xlocT_f8"] = xloc.astype(f8)
        if use_mask:
            am_sa = ((t_mask[0, 0] == 0) * MASK_NEG).astype(np.float32)
            am_ca = ((s_mask[0, 0] == 0) * MASK_NEG).astype(np.float32)
            m["amask_saT"] = np.ascontiguousarray(am_sa[rows].T).astype(bf16)
            m["amask_caT"] = np.ascontiguousarray(am_ca[rows].T).astype(bf16)
        in_maps.append(m)
    return in_maps, use_mask


def _pack_weights(kw):
    w = {}
    for src, dst in (("sa_wq", "w_saq"), ("ca_wq", "w_caq")):
        w[dst] = (np.ascontiguousarray(np.asarray(kw[src]).T) * WS).astype(f8)
    for src, dst in (("sa_wo", "w_sao"), ("ca_wo", "w_cao"), ("w1", "w1t"),
                     ("w2", "w2t")):
        w[dst] = np.ascontiguousarray(np.asarray(kw[src]).T).astype(bf16)
    w["b1r"] = np.asarray(kw["b1"]).reshape(1, -1).astype(bf16)
    gbe = [np.asarray(kw[k]).astype(np.float32).reshape(DC, 128).T for k in
           ("g1", "be1", "g2", "be2", "g3", "be3")]
    w["gbe_all"] = np.ascontiguousarray(np.concatenate(gbe, axis=1))
    w["id_t"] = np.eye(128, dtype=np.float32).astype(bf16)
    kvT = {}
    for src, nm in (("sa_wk", "w_sak"), ("sa_wv", "w_sav"),
                    ("ca_wk", "w_cak"), ("ca_wv", "w_cav")):
        kvT[nm] = (np.ascontiguousarray(np.asarray(kw[src]).T) * WS).astype(f8)
    w["_kvT"] = kvT
    w["w_sak0"] = np.ascontiguousarray(kvT["w_sak"][:, 0:KSLICE])
    w["w_sav0"] = np.ascontiguousarray(kvT["w_sav"][:, 0:KSLICE])
    w["_bias"] = {nm: np.asarray(kw[src]).reshape(-1).astype(np.float32)
                  for src, nm in (("sa_bq", "b_saq"), ("sa_bo", "b_sao"),
                                  ("ca_bq", "b_caq"), ("ca_bo", "b_cao"),
                                  ("b2", "b2r"), ("sa_bk", "b_sak"),
                                  ("sa_bv", "b_sav"), ("ca_bk", "b_cak"),
                                  ("ca_bv", "b_cav"))}
    return w


def _get_runner(use_mask):
    key = bool(use_mask)
    if key not in _CACHE:
        nc = _build(key)
        _CACHE[key] = _make_runner(nc)
    return _CACHE[key]


def kernel(dec, enc, t_mask, s_mask, timeit=False, **kw):
    dec = np.asarray(dec, dtype=np.float32)
    enc = np.asarray(enc, dtype=np.float32)
    weights = _pack_weights(kw)
    in_maps, use_mask = _prep_inputs(dec, enc, np.asarray(t_mask),
                                     np.asarray(s_mask), weights)
    run = _get_runner(use_mask)
    per_core, times = run(in_maps, timeit=timeit)
    kernel._last_per_core = per_core
    out = np.empty((B, T, D), np.float32)
    for c in range(N_CORES):
        b, s = c // CPB, c % CPB
        out[b, s * TL:(s + 1) * TL] = per_core[c]["outT"].T
    if timeit:
        kernel._last_times = times
    return out


# revision 51
# speedup vs baseline: 1.0549x; 1.0549x over previous
"""Trainium2 Bass kernel for a transformer decoder layer (self-attn + cross-attn + FFN).

Sharding: 8 cores; cores 0-3 handle batch 0, cores 4-7 batch 1; each core owns a
contiguous 512-token slice of queries.  K/V projections are head-sharded within
each batch group (256 o-dims per core) and AllGathered in fp8; additionally every
core redundantly computes K/V for heads 0-3 so the first two head-pairs of
self-attention run before the AllGather lands.

Precision: fp8e4m3 (+DoubleRow, K=256/matmul) only where the error is softmax-
attenuated — K/V/Q projections (weights x32), Q.K^T scores, exp(), and the AV
matmul.  O-projection, fc1 and fc2 run bf16 (their weight-quantization error
lands directly on the residual stream).  LayerNorm: var = E[x^2]-E[x]^2, rstd =
exp(-0.5*ln(var+eps)) so the scalar engine never leaves the exp/ln table set.
Softmax denominators ride the AV matmul as an appended ones column (output
partition 64); row broadcasts go through GPSIMD partition_broadcast; residual
adds ride PSUM as identity matmuls.  Queries run in two 256-column chunks so
chunk-0 FFN overlaps chunk-1 exps on the scalar engine.
"""

import os
import sys
import time

for _p in ("/opt/trn_rl_repo", "/root/.axon_site/_ro/trn_rl_repo"):
    if _p not in sys.path:
        sys.path.append(_p)

import numpy as np
import ml_dtypes

B, T, D, H, DH, FFN = 2, 2048, 1024, 16, 64, 4096
N_CORES = 8
CPB = N_CORES // B          # cores per batch
TL = T // CPB               # local tokens per core
DC = D // 128               # d-chunks (8)
KC = T // 128               # key chunks (16)
OC1 = FFN // 128            # fc1 out chunks (32)
NT = T // 512               # 512-wide column tiles over T
EPS = 1e-12
SCALE = 1.0 / 8.0           # 1/sqrt(DH)
MASK_NEG = -80000.0         # additive mask value (pre-scale)

W = 256                     # query chunk width
NCH = TL // W               # chunks (2)
VP = 80                     # padded V free width (64 + den col + pad, %16==0)

WS = 32.0                   # q/k/v weight fp8 pre-scale

USE_DR = not os.environ.get("BASS_NO_DR")
DEBUG_DUMP = bool(os.environ.get("BASS_DEBUG_DUMP"))

bf16 = ml_dtypes.bfloat16
f8 = ml_dtypes.float8_e4m3

_CACHE = {}

KSLICE = 2 * 128            # per-core K/V head-dim slice
KPART = KSLICE * T
VPART = T * KSLICE
AGLEN = KPART + VPART


def _emit(ctx, tc, nc, aps, use_mask, dbg=None):
    import concourse.bass as bass
    from concourse import mybir
    from contextlib import ExitStack
    dt = mybir.dt
    AF = mybir.ActivationFunctionType
    Alu = mybir.AluOpType
    PM = mybir.MatmulPerfMode
    fp32 = dt.float32
    bf = dt.bfloat16
    e4 = dt.float8e4

    consts = ctx.enter_context(tc.tile_pool(name="consts", bufs=1))
    sbA = ctx.enter_context(tc.tile_pool(name="sbA", bufs=1))
    smallp = ctx.enter_context(tc.tile_pool(name="smallp", bufs=4))
    bcp = ctx.enter_context(tc.tile_pool(name="bcp", bufs=4))
    lnp = ctx.enter_context(tc.tile_pool(name="lnp", bufs=1))
    ps_lin = ctx.enter_context(tc.tile_pool(name="ps_lin", bufs=2, space="PSUM"))
    ps_sc = ctx.enter_context(tc.tile_pool(name="ps_sc", bufs=2, space="PSUM"))
    ps_av = ctx.enter_context(tc.tile_pool(name="ps_av", bufs=2, space="PSUM"))

    # ---- constants ----
    ones_bf = consts.tile([1, 512], bf)
    nc.gpsimd.memset(ones_bf[:], 1.0)
    ones_bfc = consts.tile([128, 1], bf)
    nc.gpsimd.memset(ones_bfc[:], 1.0)
    idt = consts.tile([128, 128], bf, tag="idt")
    nc.sync.dma_start(idt[:], aps["id_t"][:])
    eps_t = consts.tile([1, 1], fp32)
    nc.gpsimd.memset(eps_t[:], EPS)

    # packed bias rows (q/k/v pre-scaled x32 on host; o/fc at true scale)
    NB = 5 * D + 6 * KSLICE
    batile = consts.tile([1, NB], bf, tag="batile")
    nc.sync.dma_start(batile[:], aps["biases_att"][:])
    _off = {}
    _o = 0
    for nm, ln in (("b_saq", D), ("b_sao", D), ("b_caq", D), ("b_cao", D),
                   ("b2r", D), ("b_sak", KSLICE), ("b_sav", KSLICE),
                   ("b_cak", KSLICE), ("b_cav", KSLICE),
                   ("b_sak0", KSLICE), ("b_sav0", KSLICE)):
        _off[nm] = (_o, ln)
        _o += ln
    bias_rows = {nm: batile[:, o:o + ln] for nm, (o, ln) in _off.items()}

    gbe_t = consts.tile([128, 48], fp32, tag="gbe")
    nc.sync.dma_start(gbe_t[:], aps["gbe_all"][:])

    def gbe(i):
        return gbe_t[:, i * DC:(i + 1) * DC]

    # =========================== helpers ===========================

    def mm2(p_out, lhsT3, rhs3, start, stop):
        """One fp8 DoubleRow matmul (or two plain ones when disabled)."""
        if USE_DR:
            nc.tensor.matmul(p_out, lhsT3, rhs3, start=start, stop=stop,
                             perf_mode=PM.DoubleRow)
        else:
            nc.tensor.matmul(p_out, lhsT3[:, 0], rhs3[:, 0],
                             start=start, stop=False)
            nc.tensor.matmul(p_out, lhsT3[:, 1], rhs3[:, 1],
                             start=False, stop=stop)

    def qproj_chain(p_out, w_sb, oc, x_sb, xsl, b_row):
        """fp8 q-projection: 4 DR matmuls + rank-1 bias."""
        for j in range(DC // 2):
            mm2(p_out, w_sb[:, 2 * j:2 * j + 2, oc * 128:(oc + 1) * 128],
                x_sb[:, 2 * j:2 * j + 2, xsl], start=(j == 0), stop=False)
        nc.tensor.matmul(p_out, b_row[:, oc * 128:(oc + 1) * 128],
                         ones_bf[:, 0:p_out.shape[-1]], start=False, stop=True)

    # =========================== K/V ===========================

    agp = ctx.enter_context(tc.tile_pool(name="agp", bufs=1, space="DRAM"))
    drp = ctx.enter_context(tc.tile_pool(name="drp", bufs=2, space="DRAM"))
    kvw = ctx.enter_context(tc.tile_pool(name="kvw", bufs=2))
    stgp = ctx.enter_context(tc.tile_pool(name="stgp", bufs=2))
    srcp = ctx.enter_context(tc.tile_pool(name="srcp", bufs=2))

    def kv_proj(src_name, wk_name, bk, wv_name, bv, k_sink, v_sink):
        """Stream src columns; project K (feature-major) and V (token-major)."""
        wk = kvw.tile([128, DC, KSLICE], e4, tag="wkv")
        nc.sync.dma_start(wk[:], aps[wk_name].rearrange("(c p) o -> p c o", p=128))
        wv = kvw.tile([128, DC, KSLICE], e4, tag="wkv")
        nc.sync.dma_start(wv[:], aps[wv_name].rearrange("(c p) o -> p c o", p=128))
        src_ap = aps[src_name].rearrange("(c p) t -> p c t", p=128)
        for nt in range(NT):
            src = srcp.tile([128, DC, 512], e4, tag="src")
            nc.sync.dma_start(src[:], src_ap[:, :, nt * 512:(nt + 1) * 512])
            for oc in range(2):
                p = ps_lin.tile([128, 512], fp32, tag="lin")
                for j in range(DC // 2):
                    mm2(p[:], wk[:, 2 * j:2 * j + 2, oc * 128:(oc + 1) * 128],
                        src[:, 2 * j:2 * j + 2, :], start=(j == 0), stop=False)
                nc.tensor.matmul(p[:], bias_rows[bk][:, oc * 128:(oc + 1) * 128],
                                 ones_bf[:], start=False, stop=True)
                k_sink(oc, nt, p)
            for k4 in range(4):
                kc = 4 * nt + k4
                p = ps_lin.tile([128, 512], fp32, tag="lin")
                for j in range(DC // 2):
                    mm2(p[:, 0:KSLICE],
                        src[:, 2 * j:2 * j + 2, k4 * 128:(k4 + 1) * 128],
                        wv[:, 2 * j:2 * j + 2, :], start=(j == 0), stop=False)
                nc.tensor.matmul(p[:, 0:KSLICE], ones_bf[:, 0:128],
                                 bias_rows[bv][:], start=False, stop=True)
                v_sink(kc, p)

    def kv_part(src_name, wk_name, bk, wv_name, bv):
        """This core's K/V head-slice (fp8) -> AllGather."""
        ag_in = agp.tile([AGLEN], e4, tag="ag_in")
        k_reg = ag_in[0:KPART].rearrange("(oc p t) -> oc p t", oc=2, p=128)
        v_reg = ag_in[KPART:AGLEN].rearrange("(kc p o) -> kc p o", kc=KC, p=128)

        def k_sink(oc, nt, p):
            stg = stgp.tile([128, 512], e4, tag="stg")
            nc.scalar.activation(out=stg[:], in_=p[:], func=AF.Copy,
                                 scale=1.0 / WS)
            nc.sync.dma_start(k_reg[oc, :, nt * 512:(nt + 1) * 512], stg[:])

        def v_sink(kc, p):
            stg = stgp.tile([128, 512], e4, tag="stg")
            nc.scalar.activation(out=stg[:, 0:KSLICE], in_=p[:, 0:KSLICE],
                                 func=AF.Copy, scale=1.0 / WS)
            nc.sync.dma_start(v_reg[kc], stg[:, 0:KSLICE])

        kv_proj(src_name, wk_name, bk, wv_name, bv, k_sink, v_sink)
        ag_out = agp.tile([CPB, AGLEN], e4, tag="ag_out")
        nc.gpsimd.collective_compute(
            "AllGather", Alu.bypass,
            ins=[ag_in.opt()], outs=[ag_out.opt()],
            replica_groups=[list(range(CPB)), list(range(CPB, 2 * CPB))])
        return ag_out

    def local_kv(src_name, wk_name, bk, wv_name, bv, lkv):
        """K/V for heads 0-3 computed locally on every core (covers pr 0,1)."""
        lktp = lkv.tile([128, 2, T], e4, tag="lktp")
        lva = lkv.tile([128, KC, 4, VP], e4, tag="lva")
        nc.gpsimd.memset(lva[:, :, :, 64:65], 1.0)
        nc.gpsimd.memset(lva[:, :, :, 65:VP], 0.0)

        def k_sink(oc, nt, p):
            nc.scalar.activation(out=lktp[:, oc, nt * 512:(nt + 1) * 512],
                                 in_=p[:], func=AF.Copy, scale=1.0 / WS)

        def v_sink(kc, p):
            nc.scalar.activation(
                out=lva[:, kc, :, 0:64],
                in_=p[:, 0:KSLICE].rearrange("p (h d) -> p h d", h=4),
                func=AF.Copy, scale=1.0 / WS)

        kv_proj(src_name, wk_name, bk, wv_name, bv, k_sink, v_sink)
        return lktp, lva

    # =========================== attention ===========================

    ktpp = ctx.enter_context(tc.tile_pool(name="ktpp", bufs=2))
    vap = ctx.enter_context(tc.tile_pool(name="vap", bufs=2))
    expp = ctx.enter_context(tc.tile_pool(name="expp", bufs=2))

    def attn_pr(qt, pr, c, ag_out, lkv_pair, mask_sb, praw, den_all, dbg=None):
        """One (head-pair, query-chunk): QK^T, exp, AV; stage raw AV + denoms."""
        csl = slice(c * W, (c + 1) * W)
        if lkv_pair is not None:
            lktp, lva = lkv_pair
            ktp_t = lktp[:, pr, :]
            va_of = {hf: lva[:, :, 2 * pr + hf, :] for hf in range(2)}
        else:
            ktp_t = ktpp.tile([128, T], e4, tag="ktp")
            k_src = ag_out[pr // 2, 0:KPART].rearrange(
                "(oc p t) -> oc p t", oc=2, p=128)[pr % 2]
            nc.sync.dma_start(ktp_t[:], k_src)
            va_of = {}
            for hf in range(2):
                h = 2 * pr + hf
                va_h = vap.tile([128, KC, VP], e4, tag="vah")
                nc.gpsimd.memset(va_h[:, :, 64:65], 1.0)
                nc.gpsimd.memset(va_h[:, :, 65:VP], 0.0)
                v_src = ag_out[h // 4, KPART:AGLEN].rearrange(
                    "(kc p hl d) -> p kc hl d", kc=KC, p=128, hl=4)
                nc.sync.dma_start(va_h[:, :, 0:64], v_src[:, :, h % 4, :])
                va_of[hf] = va_h
        exp_t = expp.tile([128, 8, 2, 2, W], e4, tag="exp")
        for g in range(8):
            p = ps_sc.tile([128, 2, 2, W], fp32, tag="sc")
            for j2 in range(2):
                kc = 2 * g + j2
                for hf in range(2):
                    po = 64 * hf
                    nc.tensor.matmul(
                        p[:, hf, j2, :],
                        ktp_t[po:po + 64, kc * 128:(kc + 1) * 128],
                        qt[po:po + 64, pr, csl], start=True, stop=True)
                    if mask_sb is not None:
                        nc.vector.tensor_add(
                            out=p[:, hf, j2, :], in0=p[:, hf, j2, :],
                            in1=mask_sb[:, kc, csl])
            nc.scalar.activation(out=exp_t[:, g], in_=p[:], func=AF.Exp,
                                 scale=SCALE)
        if dbg is not None:
            nc.sync.dma_start(dbg["d_exp"],
                              exp_t.rearrange("p g a b w -> p (g a b w)"))
        for hf in range(2):
            pav_t = ps_av.tile([128, 512], fp32, tag="av")
            pav = pav_t[0:VP, 0:W]
            for g in range(8):
                mm2(pav, va_of[hf][:, 2 * g:2 * g + 2, :], exp_t[:, g, hf],
                    start=(g == 0), stop=(g == 7))
            h = 2 * pr + hf
            po = 64 * hf
            nc.vector.tensor_copy(out=praw[po:po + 64, :], in_=pav[0:64, :])
            dtmp = smallp.tile([1, W], fp32, tag="dtmp", bufs=4)
            nc.vector.tensor_copy(out=dtmp[:], in_=pav[64:65, :])
            nc.sync.dma_start(den_all[h:h + 1, :], dtmp[:])
            if dbg is not None and hf == 0:
                dpav = smallp.tile([128, W], fp32, tag="dpav", bufs=1)
                nc.vector.tensor_copy(out=dpav[0:VP, :], in_=pav[:])
                nc.sync.dma_start(dbg["d_pav"], dpav[:])

    def finish_attn(praws, den_all, attn_dst, dbg=None):
        """Batched softmax normalize: one reciprocal for all 16 heads."""
        rec = smallp.tile([16, W], bf, tag="rec", bufs=2)
        with nc.allow_low_precision(reason="softmax denominators are O(1e3)"):
            nc.vector.reciprocal(out=rec[:], in_=den_all[:])
        rdr = drp.tile([16, W], bf, tag="rdr")
        nc.sync.dma_start(rdr[:], rec[:])
        for pr in range(DC):
            rb = bcp.tile([128, W], bf, tag="bc")
            nc.sync.dma_start(rb[0:64, :],
                              rdr[2 * pr:2 * pr + 1, :].to_broadcast([64, W]))
            nc.sync.dma_start(rb[64:128, :],
                              rdr[2 * pr + 1:2 * pr + 2, :].to_broadcast([64, W]))
            if dbg is not None and pr == 0:
                nc.sync.dma_start(dbg["d_rrow"], rec[0:1, :])
                nc.sync.dma_start(dbg["d_rb"], rb[0:64, :])
            nc.vector.tensor_mul(out=attn_dst[:, pr, :],
                                 in0=praws[pr][:], in1=rb[:])

    # ======================= bf16 linears =======================

    wop = ctx.enter_context(tc.tile_pool(name="wop", bufs=2))

    def oproj(attn_c, wname, b_o, res_bf, res_sl, c):
        """bf16 O-projection + bias + residual (identity matmul) -> y bf16."""
        w_ap = aps[wname].rearrange("(c p) o -> p c o", p=128)
        y = lnp.tile([128, DC, W], bf, tag="y", bufs=2)
        for oc in range(DC):
            wo_oc = wop.tile([128, DC, 128], bf, tag="wo")
            nc.sync.dma_start(wo_oc[:], w_ap[:, :, oc * 128:(oc + 1) * 128])
            p = ps_lin.tile([128, 512], fp32, tag="lin")
            pw = p[:, 0:W]
            for dc in range(DC):
                nc.tensor.matmul(pw, wo_oc[:, dc, :], attn_c[:, dc, :],
                                 start=(dc == 0), stop=False)
            nc.tensor.matmul(pw, b_o[:, oc * 128:(oc + 1) * 128],
                             ones_bf[:, 0:W], start=False, stop=False)
            nc.tensor.matmul(pw, idt[:], res_bf[:, oc, res_sl],
                             start=False, stop=True)
            nc.scalar.activation(out=y[:, oc, :], in_=pw, func=AF.Copy,
                                 scale=1.0)
        return y

    def layernorm(y, gi, out_bf, out_f8, out_dma):
        """LN over d: var = E[x^2]-E[x]^2, rstd = exp(-0.5*ln(var+eps))."""
        g, be = gbe(2 * gi), gbe(2 * gi + 1)
        pm_t = ps_lin.tile([128, 512], fp32, tag="lin")
        pv_t = ps_lin.tile([128, 512], fp32, tag="lin")
        pm, pv = pm_t[0:1, 0:W], pv_t[0:1, 0:W]
        for dc in range(DC):
            ysq = lnp.tile([128, W], bf, tag="ysq", bufs=3)
            nc.vector.tensor_mul(out=ysq[:], in0=y[:, dc, :], in1=y[:, dc, :])
            nc.tensor.matmul(pm, ones_bfc[:], y[:, dc, :],
                             start=(dc == 0), stop=(dc == DC - 1))
            nc.tensor.matmul(pv, ones_bfc[:], ysq[:],
                             start=(dc == 0), stop=(dc == DC - 1))
        mrow = smallp.tile([1, W], fp32, tag="row")
        nc.vector.tensor_scalar(out=mrow[:], in0=pm, scalar1=1.0 / D,
                                scalar2=None, op0=Alu.mult)
        msq = smallp.tile([1, W], fp32, tag="row")
        nc.vector.tensor_scalar(out=msq[:], in0=pv, scalar1=1.0 / D,
                                scalar2=None, op0=Alu.mult)
        var = smallp.tile([1, W], fp32, tag="row")
        nc.vector.tensor_mul(out=var[:], in0=mrow[:], in1=mrow[:])
        nc.vector.tensor_sub(out=var[:], in0=msq[:], in1=var[:])
        lnv = smallp.tile([1, W], fp32, tag="row")
        nc.scalar.activation(out=lnv[:], in_=var[:], func=AF.Ln, bias=eps_t[:])
        rstd = smallp.tile([1, W], bf, tag="rowb")
        nc.scalar.activation(out=rstd[:], in_=lnv[:], func=AF.Exp, scale=-0.5)
        mbf = smallp.tile([1, W], bf, tag="rowb")
        nc.vector.tensor_copy(out=mbf[:], in_=mrow[:])
        rb = bcp.tile([128, W], bf, tag="bcb")
        nc.gpsimd.partition_broadcast(rb[:], rstd[:])
        mb = bcp.tile([128, W], bf, tag="bcb")
        nc.gpsimd.partition_broadcast(mb[:], mbf[:])
        for dc in range(DC):
            t0 = lnp.tile([128, W], bf, tag="t0", bufs=3)
            nc.vector.tensor_sub(out=t0[:], in0=y[:, dc, :], in1=mb[:])
            nc.vector.tensor_mul(out=t0[:], in0=t0[:], in1=rb[:])
            if out_dma is not None:
                od = lnp.tile([128, W], fp32, tag="od", bufs=2)
                nc.vector.tensor_scalar(
                    out=od[:], in0=t0[:],
                    scalar1=g[:, dc:dc + 1], scalar2=be[:, dc:dc + 1],
                    op0=Alu.mult, op1=Alu.add)
                nc.sync.dma_start(out_dma[:, dc, :], od[:])
            else:
                nc.vector.tensor_scalar(
                    out=out_bf[:, dc, :], in0=t0[:],
                    scalar1=g[:, dc:dc + 1], scalar2=be[:, dc:dc + 1],
                    op0=Alu.mult, op1=Alu.add)
                if out_f8 is not None:
                    nc.vector.tensor_scalar(
                        out=out_f8[:, dc, :], in0=t0[:],
                        scalar1=g[:, dc:dc + 1], scalar2=be[:, dc:dc + 1],
                        op0=Alu.mult, op1=Alu.add)

    # =========================== pipeline ===========================

    with ExitStack() as attn_ctx:
        lkv = attn_ctx.enter_context(tc.tile_pool(name="lkv", bufs=1))
        wqp = attn_ctx.enter_context(tc.tile_pool(name="wqp", bufs=2))

        ag_sa = kv_part("decT_f8", "w_sak", "b_sak", "w_sav", "b_sav")
        ag_ca = kv_part("encT_f8", "w_cak", "b_cak", "w_cav", "b_cav")
        do_local = not use_mask
        lkv_sa = (local_kv("decT_f8", "w_sak0", "b_sak0", "w_sav0", "b_sav0",
                           lkv) if do_local else None)

        xloc_bf = sbA.tile([128, DC, TL], bf, tag="xlbf")
        nc.sync.dma_start(xloc_bf[:],
                          aps["xlocT_bf"].rearrange("(c p) t -> p c t", p=128))
        xloc_f8 = sbA.tile([128, DC, TL], e4, tag="xlf8")
        nc.sync.dma_start(xloc_f8[:],
                          aps["xlocT_f8"].rearrange("(c p) t -> p c t", p=128))

        mask_sa = mask_ca = None
        if use_mask:
            mask_sa = sbA.tile([128, KC, TL], bf, tag="mask", name="mask_sa")
            nc.sync.dma_start(mask_sa[:],
                              aps["amask_saT"].rearrange("(c p) t -> p c t", p=128))

        # SA Q projection (fp8, full width)
        wq = wqp.tile([128, DC, D], e4, tag="wq")
        nc.sync.dma_start(wq[:], aps["w_saq"].rearrange("(c p) o -> p c o", p=128))
        qt_sa = sbA.tile([128, DC, TL], e4, tag="qt")
        for oc in range(DC):
            p = ps_lin.tile([128, 512], fp32, tag="lin")
            qproj_chain(p[:], wq, oc, xloc_f8, slice(None), bias_rows["b_saq"])
            nc.scalar.activation(out=qt_sa[:, oc, :], in_=p[:], func=AF.Copy,
                                 scale=1.0 / WS)
        if dbg is not None:
            nc.sync.dma_start(dbg["d_qt"],
                              qt_sa.rearrange("p c t -> p (c t)"))
            if lkv_sa is not None:
                nc.sync.dma_start(dbg["d_ktp"],
                                  lkv_sa[0].rearrange("p c t -> p (c t)"))

        attn_sa = [sbA.tile([128, DC, W], bf, tag=f"attn{c}", name=f"attnsa{c}")
                   for c in range(NCH)]
        den_sa = [smallp.tile([16, W], fp32, tag="den", bufs=4,
                              name=f"densa{c}") for c in range(NCH)]
        praw_sa = [[None] * DC for _ in range(NCH)]

        def new_praw(store, c, pr):
            t = lnp.tile([128, W], bf, tag="praw", bufs=11,
                         name=f"praw{c}_{pr}")
            store[c][pr] = t
            return t

        if do_local:
            for pr in range(2):
                for c in range(NCH):
                    attn_pr(qt_sa, pr, c, None, lkv_sa, mask_sa,
                            new_praw(praw_sa, c, pr), den_sa[c],
                            dbg=(dbg if (pr == 0 and c == 0) else None))

        wqc = wqp.tile([128, DC, D], e4, tag="wq")
        nc.sync.dma_start(wqc[:], aps["w_caq"].rearrange("(c p) o -> p c o", p=128))
        qt_ca = sbA.tile([128, DC, TL], e4, tag="qtc")
        x1_bf = [sbA.tile([128, DC, W], bf, tag=f"x1b{c}", name=f"x1b{c}")
                 for c in range(NCH)]
        x1_f8 = [sbA.tile([128, DC, W], e4, tag=f"x1f{c}", name=f"x1f{c}")
                 for c in range(NCH)]

        pr0 = 0 if not do_local else 2
        for c in range(NCH):
            for pr in range(pr0, DC):
                attn_pr(qt_sa, pr, c, ag_sa, None, mask_sa,
                        new_praw(praw_sa, c, pr), den_sa[c])
            finish_attn(praw_sa[c], den_sa[c], attn_sa[c],
                        dbg=(dbg if c == 0 else None))
            y1 = oproj(attn_sa[c], "w_sao", bias_rows["b_sao"], xloc_bf,
                       slice(c * W, (c + 1) * W), c)
            if dbg is not None and c == 0:
                nc.sync.dma_start(dbg["d_attn"],
                                  attn_sa[0].rearrange("p c t -> p (c t)"))
                nc.sync.dma_start(dbg["d_y1"],
                                  y1.rearrange("p c t -> p (c t)"))
            layernorm(y1, 0, x1_bf[c], x1_f8[c], None)
            if dbg is not None and c == 0:
                nc.sync.dma_start(dbg["d_x1"],
                                  x1_bf[0].rearrange("p c t -> p (c t)"))
            for oc in range(DC):
                p = ps_lin.tile([128, 512], fp32, tag="lin")
                qproj_chain(p[:, 0:W], wqc, oc, x1_f8[c], slice(None),
                            bias_rows["b_caq"])
                nc.scalar.activation(out=qt_ca[:, oc, c * W:(c + 1) * W],
                                     in_=p[:, 0:W], func=AF.Copy, scale=1.0 / WS)

        # ---- cross attention + FFN, chunk-pipelined ----
        if use_mask:
            mask_ca = sbA.tile([128, KC, TL], bf, tag="mask", name="mask_ca")
            nc.sync.dma_start(mask_ca[:],
                              aps["amask_caT"].rearrange("(c p) t -> p c t", p=128))
        w1p = attn_ctx.enter_context(tc.tile_pool(name="w1p", bufs=4))
        w2p = attn_ctx.enter_context(tc.tile_pool(name="w2p", bufs=2))
        w1_ap = aps["w1t"].rearrange("(c p) o -> p c o", p=128)
        w2_ap = aps["w2t"].rearrange("(c p) o -> p c o", p=128)
        b1r = consts.tile([1, FFN], bf, tag="b1r")
        nc.sync.dma_start(b1r[:], aps["b1r"][:])

        attn_ca = [sbA.tile([128, DC, W], bf, tag=f"attn{c}", name=f"attnca{c}")
                   for c in range(NCH)]
        x2_bf = [sbA.tile([128, DC, W], bf, tag=f"x2b{c}", name=f"x2b{c}")
                 for c in range(NCH)]
        out_ap = aps["outT"].rearrange("(c p) t -> p c t", p=128)

        def fc1_part(ht, c, ocs):
            for oc in ocs:
                w1oc = w1p.tile([128, DC, 128], bf, tag="w1")
                nc.sync.dma_start(w1oc[:], w1_ap[:, :, oc * 128:(oc + 1) * 128])
                p = ps_lin.tile([128, 512], fp32, tag="lin")
                pw = p[:, 0:W]
                for dc in range(DC):
                    nc.tensor.matmul(pw, w1oc[:, dc, :], x2_bf[c][:, dc, :],
                                     start=(dc == 0), stop=False)
                nc.tensor.matmul(pw, b1r[:, oc * 128:(oc + 1) * 128],
                                 ones_bf[:, 0:W], start=False, stop=True)
                nc.scalar.activation(out=ht[:, oc, :], in_=pw, func=AF.Relu,
                                     scale=1.0)

        def ffn_chunk(c, ht=None, skip_fc1=False):
            if ht is None:
                ht = lnp.tile([128, OC1, W], bf, tag="ht", bufs=1)
            if not skip_fc1:
                fc1_part(ht, c, range(OC1))
            y3 = lnp.tile([128, DC, W], bf, tag="y", bufs=2)
            for oc in range(DC):
                for half in range(2):
                    w2oc = w2p.tile([128, OC1 // 2, 128], bf, tag="w2")
                    nc.sync.dma_start(
                        w2oc[:],
                        w2_ap[:, half * 16:(half + 1) * 16,
                              oc * 128:(oc + 1) * 128])
                    if half == 0:
                        p = ps_lin.tile([128, 512], fp32, tag="lin")
                        pw = p[:, 0:W]
                    for j in range(OC1 // 2):
                        nc.tensor.matmul(
                            pw, w2oc[:, j, :], ht[:, half * 16 + j, :],
                            start=(half == 0 and j == 0), stop=False)
                nc.tensor.matmul(pw, bias_rows["b2r"][:, oc * 128:(oc + 1) * 128],
                                 ones_bf[:, 0:W], start=False, stop=False)
                nc.tensor.matmul(pw, idt[:], x2_bf[c][:, oc, :],
                                 start=False, stop=True)
                nc.scalar.activation(out=y3[:, oc, :], in_=pw, func=AF.Copy,
                                     scale=1.0)
            layernorm(y3, 2, None, None, out_ap[:, :, c * W:(c + 1) * W])

        den_ca = [smallp.tile([16, W], fp32, tag="den", bufs=4,
                              name=f"denca{c}") for c in range(NCH)]
        praw_ca = [[None] * DC for _ in range(NCH)]
        # chunk 0
        for pr in range(DC):
            attn_pr(qt_ca, pr, 0, ag_ca, None, mask_ca,
                    new_praw(praw_ca, 0, pr), den_ca[0])
        finish_attn(praw_ca[0], den_ca[0], attn_ca[0])
        y2 = oproj(attn_ca[0], "w_cao", bias_rows["b_cao"], x1_bf[0],
                   slice(None), 0)
        layernorm(y2, 1, x2_bf[0], None, None)
        # chunk 1 attention with chunk-0 fc1 matmuls interleaved so the PE
        # chews FFN work while the scalar engine runs chunk-1 exps
        ht0 = lnp.tile([128, OC1, W], bf, tag="ht", bufs=1)
        for pr in range(DC):
            attn_pr(qt_ca, pr, 1, ag_ca, None, mask_ca,
                    new_praw(praw_ca, 1, pr), den_ca[1])
            fc1_part(ht0, 0, range(4 * pr, 4 * pr + 4))
        finish_attn(praw_ca[1], den_ca[1], attn_ca[1])
        ffn_chunk(0, ht=ht0, skip_fc1=True)
        y2 = oproj(attn_ca[1], "w_cao", bias_rows["b_cao"], x1_bf[1],
                   slice(None), 1)
        layernorm(y2, 1, x2_bf[1], None, None)
        ffn_chunk(1)


def _patch_act_tables():
    """Restrict the activation-table chooser to the exp+ln set so the scalar
    engine never reloads tables mid-kernel (we only use Exp/Ln/Copy/Relu)."""
    import concourse.hw_specs as hw_specs
    if getattr(hw_specs, "_ant_tables_patched", False):
        return
    orig = hw_specs.get_activation_tables

    def patched(arch):
        tabs = dict(orig(arch))
        keep = "natural_log_exp_and_others"
        if keep in tabs:
            tabs = {k: (v if k == keep else type(v)()) for k, v in tabs.items()}
        return tabs

    patched.cache_clear = getattr(orig, "cache_clear", lambda: None)
    hw_specs.get_activation_tables = patched
    import concourse.bacc as _bacc
    _bacc.get_activation_tables = patched
    hw_specs._ant_tables_patched = True


def _build(use_mask):
    import concourse.bass as bass
    import concourse.tile as tile
    from concourse import bacc, mybir
    _patch_act_tables()
    dt = mybir.dt
    nc = bacc.Bacc("TRN2", target_bir_lowering=False, debug=False,
                   num_devices=N_CORES)
    aps = {}

    def inp(name, shape, dtype):
        aps[name] = nc.dram_tensor(name, shape, dtype, kind="ExternalInput").ap()

    inp("decT_f8", [D, T], dt.float8e4)
    inp("encT_f8", [D, T], dt.float8e4)
    inp("xlocT_bf", [D, TL], dt.bfloat16)
    inp("xlocT_f8", [D, TL], dt.float8e4)
    for nm in ("w_saq", "w_caq"):
        inp(nm, [D, D], dt.float8e4)
    for nm in ("w_sao", "w_cao"):
        inp(nm, [D, D], dt.bfloat16)
    for nm in ("w_sak", "w_sav", "w_cak", "w_cav", "w_sak0", "w_sav0"):
        inp(nm, [D, KSLICE], dt.float8e4)
    inp("w1t", [D, FFN], dt.bfloat16)
    inp("w2t", [FFN, D], dt.bfloat16)
    inp("biases_att", [1, 5 * D + 6 * KSLICE], dt.bfloat16)
    inp("b1r", [1, FFN], dt.bfloat16)
    inp("gbe_all", [128, 48], dt.float32)
    inp("id_t", [128, 128], dt.bfloat16)
    if use_mask:
        inp("amask_saT", [T, TL], dt.bfloat16)
        inp("amask_caT", [T, TL], dt.bfloat16)
    aps["outT"] = nc.dram_tensor("outT", [D, TL], dt.float32,
                                 kind="ExternalOutput").ap()
    dbg = None
    if DEBUG_DUMP:
        dbg = {}
        for nm, shape, dty in (
                ("d_qt", [128, DC * TL], dt.float8e4),
                ("d_ktp", [128, 2 * T], dt.float8e4),
                ("d_exp", [128, 8 * 4 * W], dt.float8e4),
                ("d_pav", [128, W], dt.float32),
                ("d_rrow", [1, W], dt.bfloat16),
                ("d_rb", [64, W], dt.bfloat16),
                ("d_attn", [128, DC * W], dt.bfloat16),
                ("d_y1", [128, DC * W], dt.bfloat16),
                ("d_x1", [128, DC * W], dt.bfloat16)):
            dbg[nm] = nc.dram_tensor(nm, shape, dty, kind="ExternalOutput").ap()

    from contextlib import ExitStack
    with tile.TileContext(nc) as tc:
        with ExitStack() as ctx:
            _emit(ctx, tc, nc, aps, use_mask, dbg=dbg)
    nc.compile()
    return nc


def _make_runner(nc):
    import jax
    from jax.sharding import Mesh, PartitionSpec
    from jax.experimental.shard_map import shard_map
    from concourse import bass2jax, mybir

    bass2jax.install_neuronx_cc_hook()
    part_name = nc.partition_id_tensor.name if nc.partition_id_tensor else None
    in_names, out_names, out_avals = [], [], []
    for alloc in nc.m.functions[0].allocations:
        if not isinstance(alloc, mybir.MemoryLocationSet):
            continue
        name = alloc.memorylocations[0].name
        if alloc.kind == "ExternalInput":
            if name != part_name:
                in_names.append(name)
        elif alloc.kind == "ExternalOutput":
            out_names.append(name)
            out_avals.append(jax.core.ShapedArray(tuple(alloc.tensor_shape),
                                                  mybir.dt.np(alloc.dtype)))
    n_params = len(in_names)
    all_names = list(in_names + out_names)
    if part_name is not None:
        all_names.append(part_name)
    all_names = tuple(all_names)

    def _body(*args):
        operands = list(args)
        if part_name is not None:
            operands.append(bass2jax.partition_id_tensor())
        return tuple(bass2jax._bass_exec_p.bind(
            *operands, out_avals=tuple(out_avals), in_names=all_names,
            out_names=tuple(out_names), lowering_input_output_aliases=(),
            sim_require_finite=True, sim_require_nnan=True, nc=nc))

    devices = jax.devices()[:N_CORES]
    mesh = Mesh(np.asarray(devices), ("core",))
    spec = (PartitionSpec("core"),)
    nin = n_params + len(out_names)
    donate = () if os.environ.get("BASS_KERNEL_SIM") else tuple(
        range(n_params, nin))
    jfn = jax.jit(
        shard_map(_body, mesh=mesh, in_specs=spec * nin,
                  out_specs=spec * len(out_names), check_rep=False),
        donate_argnums=donate, keep_unused=True)

    from jax.sharding import NamedSharding
    shard = NamedSharding(mesh, PartitionSpec("core"))

    def run(in_maps, timeit=False):
        concat_in = [np.concatenate([np.asarray(in_maps[c][n]) for c in range(N_CORES)],
                                    axis=0) for n in in_names]
        zeros = [np.zeros((N_CORES * a.shape[0],) + tuple(a.shape[1:]), a.dtype)
                 for a in out_avals]
        dev_in = [jax.device_put(a, shard) for a in concat_in]
        out = jfn(*dev_in, *[jax.device_put(z, shard) for z in zeros])
        jax.block_until_ready(out)
        times = []
        if timeit:
            for _rep in range(2):
                totals = {}
                for k in (4, 44):
                    zsets = [[jax.device_put(z, shard) for z in zeros]
                             for _ in range(k)]
                    jax.block_until_ready(zsets)
                    t0 = time.perf_counter()
                    outs = [jfn(*dev_in, *zs) for zs in zsets]
                    jax.block_until_ready(outs)
                    totals[k] = time.perf_counter() - t0
                times.append((totals[44] - totals[4]) / 40.0)
        per_core = [{n: np.asarray(out[i]).reshape(N_CORES, *out_avals[i].shape)[c]
                     for i, n in enumerate(out_names)} for c in range(N_CORES)]
        return per_core, times

    return run


def _prep_inputs(dec, enc, t_mask, s_mask, weights):
    use_mask_sa = not bool(np.all(t_mask != 0))
    use_mask_ca = not bool(np.all(s_mask != 0))
    use_mask = use_mask_sa or use_mask_ca

    kvT = weights["_kvT"]
    bias = weights["_bias"]
    shared = {k: v for k, v in weights.items() if not k.startswith("_")}
    in_maps = []
    for c in range(N_CORES):
        b, s = c // CPB, c % CPB
        rows = slice(s * TL, (s + 1) * TL)
        osl = slice(s * KSLICE, (s + 1) * KSLICE)
        m = dict(shared)
        for nm in ("w_sak", "w_sav", "w_cak", "w_cav"):
            m[nm] = np.ascontiguousarray(kvT[nm][:, osl])
        batt = np.concatenate([
            bias["b_saq"] * WS, bias["b_sao"], bias["b_caq"] * WS,
            bias["b_cao"], bias["b2r"],
            bias["b_sak"][osl] * WS, bias["b_sav"][osl] * WS,
            bias["b_cak"][osl] * WS, bias["b_cav"][osl] * WS,
            bias["b_sak"][0:KSLICE] * WS, bias["b_sav"][0:KSLICE] * WS])
        m["biases_att"] = batt.reshape(1, -1).astype(bf16)
        decT = np.ascontiguousarray(dec[b].T)
        m["decT_f8"] = decT.astype(f8)
        m["encT_f8"] = np.ascontiguousarray(enc[b].T).astype(f8)
        xloc = np.ascontiguousarray(dec[b, rows].T)
        m["xlocT_bf"] = xloc.astype(bf16)
        m["xlocT_f8"] = xloc.astype(f8)
        if use_mask:
            am_sa = ((t_mask[0, 0] == 0) * MASK_NEG).astype(np.float32)
            am_ca = ((s_mask[0, 0] == 0) * MASK_NEG).astype(np.float32)
            m["amask_saT"] = np.ascontiguousarray(am_sa[rows].T).astype(bf16)
            m["amask_caT"] = np.ascontiguousarray(am_ca[rows].T).astype(bf16)
        in_maps.append(m)
    return in_maps, use_mask


def _pack_weights(kw):
    w = {}
    for src, dst in (("sa_wq", "w_saq"), ("ca_wq", "w_caq")):
        w[dst] = (np.ascontiguousarray(np.asarray(kw[src]).T) * WS).astype(f8)
    for src, dst in (("sa_wo", "w_sao"), ("ca_wo", "w_cao"), ("w1", "w1t"),
                     ("w2", "w2t")):
        w[dst] = np.ascontiguousarray(np.asarray(kw[src]).T).astype(bf16)
    w["b1r"] = np.asarray(kw["b1"]).reshape(1, -1).astype(bf16)
    gbe = [np.asarray(kw[k]).astype(np.float32).reshape(DC, 128).T for k in
           ("g1", "be1", "g2", "be2", "g3", "be3")]
    w["gbe_all"] = np.ascontiguousarray(np.concatenate(gbe, axis=1))
    w["id_t"] = np.eye(128, dtype=np.float32).astype(bf16)
    kvT = {}
    for src, nm in (("sa_wk", "w_sak"), ("sa_wv", "w_sav"),
                    ("ca_wk", "w_cak"), ("ca_wv", "w_cav")):
        kvT[nm] = (np.ascontiguousarray(np.asarray(kw[src]).T) * WS).astype(f8)
    w["_kvT"] = kvT
    w["w_sak0"] = np.ascontiguousarray(kvT["w_sak"][:, 0:KSLICE])
    w["w_sav0"] = np.ascontiguousarray(kvT["w_sav"][:, 0:KSLICE])
    w["_bias"] = {nm: np.asarray(kw[src]).reshape(-1).astype(np.float32)
                  for src, nm in (("sa_bq", "b_saq"), ("sa_bo", "b_sao"),
                                  ("ca_bq", "b_caq"), ("ca_bo", "b_cao"),
                                  ("b2", "b2r"), ("sa_bk", "b_sak"),
                                  ("sa_bv", "b_sav"), ("ca_bk", "b_cak"),
                                  ("ca_bv", "b_cav"))}
    return w


def _get_runner(use_mask):
    key = bool(use_mask)
    if key not in _CACHE:
        nc = _build(key)
        _CACHE[key] = _make_runner(nc)
    return _CACHE[key]


def kernel(dec, enc, t_mask, s_mask, timeit=False, **kw):
    dec = np.asarray(dec, dtype=np.float32)
    enc = np.asarray(enc, dtype=np.float32)
    weights = _pack_weights(kw)
    in_maps, use_mask = _prep_inputs(dec, enc, np.asarray(t_mask),
                                     np.asarray(s_mask), weights)
    run = _get_runner(use_mask)
    per_core, times = run(in_maps, timeit=timeit)
    kernel._last_per_core = per_core
    out = np.empty((B, T, D), np.float32)
    for c in range(N_CORES):
        b, s = c // CPB, c % CPB
        out[b, s * TL:(s + 1) * TL] = per_core[c]["outT"].T
    if timeit:
        kernel._last_times = times
    return out
